# revision 5
# baseline (speedup 1.0000x reference)
"""Trainium2 Bass kernel v2 for nn_ChunkedCrossAttention_85907935855128.

Self-contained: hardcodes shapes/sharding. Accepts FULL inputs, returns FULL
output. Shards the fused (b*k_chunks) chunk axis across 8 NeuronCores.

v2 design vs v1 (625us -> 363us timeline-sim):
- All activations/weights DMA'd as bf16 (halves HBM traffic).
- rope-k never materialized: sim = (cos*k)^T q + (g*k)^T q_rho accumulated in
  PSUM, where the cos/g muls read the k-projection PSUM directly on all 128
  partitions (head-pair layout), and q_rho is an unsigned half-swap of q made
  by small SBUF->SBUF DMAs. Kills the perm matmuls, kraw copies and rope add.
- rope-q (affects only token 0 of each chunk after the causal shift) is folded
  into the host: x~_0 = x_0 + Wq (Wq^T Wq)^-1 (R0 - I) Wq^T x_0, so on-device
  q = Wq^T x~ is already roped.
- null-k sims are 8 extra columns of Wq (host: Wq_h @ null_k_h); one [8,256]
  exp per group; the null term of the o-matmul opens each PSUM bank's
  accumulation group via a block-diagonal null-v (4 heads per matmul).
- sim PSUM banks are parity-pure (bank = h%2): mixing PE tile_position row-0
  and row-64 matmuls within one PSUM bank crashes the device.
- software pipeline per pair: kprojA | ropeA muls | trans(prev) | kprojB |
  ropeB muls | outproj(prev) | v0 s0 v1 s1 | drain(prev) | o0 o1; o-matmuls
  deferred past both exps; out DMA and input DMAs on separate DGE queues.
- PSUM: shared 2-bank pool bufs=3 (6 banks) + otr (1) + null (1) = 8 banks.
"""
import os
# bass2jax executes via the axon PJRT platform; a CPU pin would hide the cores.
if os.environ.get("JAX_PLATFORMS", "") in ("cpu",):
    del os.environ["JAX_PLATFORMS"]

import numpy as np
import ml_dtypes

import concourse.bacc as bacc
import concourse.bass as bass
import concourse.mybir as mybir
import concourse.tile as tile
from concourse.bass_utils import run_bass_kernel_spmd
from concourse.masks import make_identity

F32 = mybir.dt.float32
BF16 = mybir.dt.bfloat16
BF = ml_dtypes.bfloat16

CS, CP, H, DH = 64, 63, 8, 64
SCALE = DH ** -0.5
N_CORES = 8
B, N, DIM = 4, 4096, 1024
K_CHUNKS, R, RLEN = 64, 2, 128
TK = R * RLEN                 # 256 ctx tokens / chunk
BK = B * K_CHUNKS             # 256 chunks
CPC = BK // N_CORES           # 32 chunks / core
TQ = CPC * CS                 # 2048 q tokens / core
TCTX = CPC * TK               # 8192 ctx tokens / core
INNER = H * DH                # 512
NP_ = CPC // 2                # 16 pairs / core
NG = CPC // 4                 # 8 groups / core (4 chunks each)


def _build_bass(cpc=CPC, num_devices=N_CORES, stage=9, force_base0=False):
    tq = cpc * CS
    tctx = cpc * TK
    ng = cpc // 4
    nc = bacc.Bacc("TRN2", target_bir_lowering=False, debug=False,
                   num_devices=num_devices)

    xT = nc.dram_tensor("xT", (DIM, tq), BF16, kind="ExternalInput")
    ctxT = nc.dram_tensor("ctxT", (DIM, tctx), BF16, kind="ExternalInput")
    Wq = nc.dram_tensor("Wq", (DIM, INNER + 8), BF16, kind="ExternalInput")
    Wk = nc.dram_tensor("Wk", (DIM, INNER), BF16, kind="ExternalInput")
    Wv = nc.dram_tensor("Wv", (DIM, INNER), BF16, kind="ExternalInput")
    Wo = nc.dram_tensor("Wo", (INNER, DIM), BF16, kind="ExternalInput")
    bo = nc.dram_tensor("bo", (DIM,), F32, kind="ExternalInput")
    Tcos = nc.dram_tensor("Tcos", (128, 128), F32, kind="ExternalInput")
    Tg = nc.dram_tensor("Tg", (128, 128), F32, kind="ExternalInput")
    nullv_aug = nc.dram_tensor("nullv_aug", (8, 520), F32, kind="ExternalInput")
    out = nc.dram_tensor("out", (tq, DIM), F32, kind="ExternalOutput")

    with tile.TileContext(nc) as tc:
        with tc.tile_pool(name="consts", bufs=1) as cp_, \
             tc.tile_pool(name="wk", bufs=2) as wk, \
             tc.tile_pool(name="psb", bufs=3, space="PSUM") as psb, \
             tc.tile_pool(name="pst", bufs=1, space="PSUM") as pst, \
             tc.tile_pool(name="psn", bufs=1, space="PSUM") as psn:

            # first-iteration input prefetch ahead of the weight DMAs
            xg0 = wk.tile([128, 8, 4 * CS], BF16, tag="xg", bufs=3,
                          name="xg_pre")
            nc.gpsimd.dma_start(out=xg0, in_=xT[:, :].rearrange(
                "(dt p) t -> p dt t", p=128)[:, :, 0:4 * CS])
            cg0 = wk.tile([128, 8, 2 * TK], BF16, tag="cg", bufs=3,
                          name="cg_pre")
            nc.gpsimd.dma_start(out=cg0, in_=ctxT[:, :].rearrange(
                "(dt p) t -> p dt t", p=128)[:, :, 0:2 * TK])

            # ---- constants ----
            wq_sb = cp_.tile([128, 8, INNER + 8], BF16)
            nc.sync.dma_start(out=wq_sb, in_=Wq[:, :].rearrange(
                "(dt p) i -> p dt i", p=128))
            wk_sb = cp_.tile([128, 8, INNER], BF16)
            nc.sync.dma_start(out=wk_sb, in_=Wk[:, :].rearrange(
                "(dt p) i -> p dt i", p=128))
            wv_sb = cp_.tile([128, 8, INNER], BF16)
            nc.sync.dma_start(out=wv_sb, in_=Wv[:, :].rearrange(
                "(dt p) i -> p dt i", p=128))
            wo_sb = cp_.tile([128, 4, DIM], BF16)
            nc.sync.dma_start(out=wo_sb, in_=Wo[:, :].rearrange(
                "(et p) c -> p et c", p=128))
            bo_bc = cp_.tile([128, DIM], F32)
            nc.sync.dma_start(out=bo_bc, in_=bass.AP(
                tensor=bo, offset=0, ap=[[0, 128], [1, DIM]]))

            tcos = cp_.tile([128, 128], F32)
            nc.sync.dma_start(out=tcos, in_=Tcos[:, :])
            tg = cp_.tile([128, 128], F32)
            nc.sync.dma_start(out=tg, in_=Tg[:, :])

            nullv_f32 = cp_.tile([8, 2, 260], F32)
            nc.sync.dma_start(out=nullv_f32, in_=nullv_aug[:, :].rearrange(
                "h (hb w) -> h hb w", hb=2))
            nullv_bf = cp_.tile([8, 2, 260], BF16)
            nc.vector.tensor_copy(nullv_bf, nullv_f32)

            ident = cp_.tile([128, 128], F32)
            make_identity(nc, ident)
            ident_bf = cp_.tile([128, 128], BF16)
            nc.vector.tensor_copy(ident_bf, ident)

            # per-pair state carried across iterations for the deferred
            # transpose/out-projection of the previous pair
            prev = {}
            pending_out = []

            def emit_trans(pv):
                # transpose o_pair(prev) -> otr; oT copy on ACT (not DVE --
                # DVE is busy with the rope muls right now)
                o_pair, p = pv["o_pair"], pv["p"]
                otr = pst.tile([128, 4, 128], BF16, tag="otr", name=f"otr{p}")
                for et in range(4):
                    nc.tensor.transpose(
                        otr[:, et, :], o_pair[:, 2 * et:2 * et + 2, :], ident_bf)
                oT_sb = wk.tile([128, 4, 128], BF16, tag="oT", bufs=2)
                nc.scalar.copy(oT_sb, otr)
                pv["oT_sb"] = oT_sb

            def emit_out(pv):
                # out-projection + bias into PSUM (drain deferred)
                oT_sb, p = pv["oT_sb"], pv["p"]
                outps = psb.tile([128, 2, 512], F32, tag="ps",
                                 name=f"outps{p}")
                for co in range(2):
                    for et in range(4):
                        nc.tensor.matmul(
                            outps[:, co, :],
                            oT_sb[:, et, :],
                            wo_sb[:, et, co * 512:(co + 1) * 512],
                            start=(et == 0), stop=(et == 3))
                pv["outps"] = outps

            def emit_drain(pv):
                # PSUM -> SBUF -> DRAM for the finished pair
                outps, p = pv["outps"], pv["p"]
                out_sb = wk.tile([128, 2, 512], F32, tag="out_sb", bufs=2)
                nc.vector.tensor_add(
                    out_sb, outps,
                    bo_bc[:, :].rearrange("p (co c) -> p co c", co=2))
                nc.sync.dma_start(
                    out=out[p * 2 * CS:(p + 1) * 2 * CS, :],
                    in_=out_sb[:, :, :].rearrange("p co c -> p (co c)"))

            for g in range(ng):
                # ---- q projection for group g (4 chunks, 256 tokens) ----
                if g == 0:
                    xg = xg0
                else:
                    xg = wk.tile([128, 8, 4 * CS], BF16, tag="xg", bufs=3)
                    nc.gpsimd.dma_start(out=xg, in_=xT[:, :].rearrange(
                        "(dt p) t -> p dt t", p=128)
                        [:, :, g * 4 * CS:(g + 1) * 4 * CS])
                qps = psb.tile([128, 4, 256], F32, tag="ps", name=f"qps{g}")
                for it in range(4):
                    for dt in range(8):
                        nc.tensor.matmul(
                            qps[:, it, :],
                            wq_sb[:, dt, it * 128:(it + 1) * 128],
                            xg[:, dt, :],
                            start=(dt == 0), stop=(dt == 7))
                qn = psn.tile([8, 256], F32, tag="qn", name=f"qn{g}")
                for dt in range(8):
                    nc.tensor.matmul(
                        qn[:, :],
                        wq_sb[:, dt, INNER:INNER + 8],
                        xg[:, dt, :],
                        start=(dt == 0), stop=(dt == 7))
                # drain q PSUM
                q_sb = wk.tile([128, 4, 256], BF16, tag="q", bufs=2)
                nc.vector.tensor_copy(q_sb, qps)
                expn8 = wk.tile([8, 256], BF16, tag="expn8", bufs=2)
                nc.scalar.activation(expn8, qn,
                                     mybir.ActivationFunctionType.Exp)

                if stage == 0:
                    # debug: write q_sb rows
                    dbg = wk.tile([128, DIM], F32, tag="dbg", bufs=2)
                    nc.vector.tensor_copy(dbg[:, 0:1024], q_sb[:, :, :].rearrange(
                        "p it t -> p (it t)"))
                    nc.sync.dma_start(out=out[g * 128:(g + 1) * 128, :], in_=dbg)
                    continue
                # q_rho: unsigned 32-partition half-swap of q (both halves)
                qr_sb = wk.tile([128, 4, 256], BF16, tag="qr", bufs=2)
                for (d, s) in ((0, 32), (32, 0), (64, 96), (96, 64)):
                    nc.gpsimd.dma_start(out=qr_sb[d:d + 32, :, :],
                                      in_=q_sb[s:s + 32, :, :])

                for pp in range(2):
                    p = g * 2 + pp            # pair index
                    # ---- ctx DMA for this pair ----
                    if p == 0:
                        cg = cg0
                    else:
                        cg = wk.tile([128, 8, 2 * TK], BF16, tag="cg", bufs=3)
                        nc.gpsimd.dma_start(out=cg, in_=ctxT[:, :].rearrange(
                            "(dt p) t -> p dt t", p=128)
                            [:, :, p * 2 * TK:(p + 1) * 2 * TK])

                    # ---- k projection (heads 0-3 then 4-7) ----
                    kpsA = psb.tile([128, 2, 2 * TK], F32, tag="ps",
                                    name=f"kpsA{p}")
                    for it in range(2):
                        for dt in range(8):
                            nc.tensor.matmul(
                                kpsA[:, it, :],
                                wk_sb[:, dt, it * 128:(it + 1) * 128],
                                cg[:, dt, :],
                                start=(dt == 0), stop=(dt == 7))
                    # rope muls for A while PE proceeds
                    kcA = wk.tile([128, 2, 2 * TK], BF16, tag="kcA", bufs=2)
                    kgA = wk.tile([128, 2, 2 * TK], BF16, tag="kgA", bufs=2)
                    kv_ = kpsA[:, :, :].rearrange(
                        "p it (rep j) -> p it rep j", j=128)
                    nc.vector.tensor_mul(
                        kcA[:, :, :].rearrange("p it (rep j) -> p it rep j", j=128),
                        kv_,
                        tcos.unsqueeze(1).unsqueeze(2).broadcast_to((128, 2, 4, 128)))
                    nc.vector.tensor_mul(
                        kgA[:, :, :].rearrange("p it (rep j) -> p it rep j", j=128),
                        kv_,
                        tg.unsqueeze(1).unsqueeze(2).broadcast_to((128, 2, 4, 128)))

                    if prev:
                        emit_trans(prev)

                    kpsB = psb.tile([128, 2, 2 * TK], F32, tag="ps",
                                    name=f"kpsB{p}")
                    for it in range(2, 4):
                        for dt in range(8):
                            nc.tensor.matmul(
                                kpsB[:, it - 2, :],
                                wk_sb[:, dt, it * 128:(it + 1) * 128],
                                cg[:, dt, :],
                                start=(dt == 0), stop=(dt == 7))
                    kcB = wk.tile([128, 2, 2 * TK], BF16, tag="kcB", bufs=2)
                    kgB = wk.tile([128, 2, 2 * TK], BF16, tag="kgB", bufs=2)
                    kv_ = kpsB[:, :, :].rearrange(
                        "p it (rep j) -> p it rep j", j=128)
                    nc.vector.tensor_mul(
                        kcB[:, :, :].rearrange("p it (rep j) -> p it rep j", j=128),
                        kv_,
                        tcos.unsqueeze(1).unsqueeze(2).broadcast_to((128, 2, 4, 128)))
                    nc.vector.tensor_mul(
                        kgB[:, :, :].rearrange("p it (rep j) -> p it rep j", j=128),
                        kv_,
                        tg.unsqueeze(1).unsqueeze(2).broadcast_to((128, 2, 4, 128)))

                    if stage == 1:
                        dbg = wk.tile([128, DIM], F32, tag="dbg", bufs=2)
                        nc.vector.tensor_copy(dbg[:, 0:1024], kcA[:, :, :]
                                              .rearrange("p it t -> p (it t)"))
                        nc.sync.dma_start(out=out[p * 128:(p + 1) * 128, :],
                                          in_=dbg)
                        continue

                    # deferred out-projection of the previous pair (PE busy
                    # while DVE finishes the rope muls above)
                    if prev:
                        emit_out(prev)
                        pending_out.append(dict(prev))
                        prev.clear()

                    if stage >= 3:
                        o_pair = wk.tile([128, 8, CS], BF16, tag="o_pair",
                                         bufs=2, name=f"o_pair{p}")
                    else:
                        o_pair = None

                    sub_state = []
                    for sub in range(2):
                        c = p * 2 + sub       # chunk index
                        cing = pp * 2 + sub   # chunk within group (0..3)
                        # ---- v projection ----
                        vps = psb.tile([128, 2, INNER], F32, tag="ps",
                                       name=f"vps{c}")
                        for tgi in range(2):
                            for dt in range(8):
                                nc.tensor.matmul(
                                    vps[:, tgi, :],
                                    cg[:, dt, sub * TK + tgi * 128:
                                       sub * TK + (tgi + 1) * 128],
                                    wv_sb[:, dt, :],
                                    start=(dt == 0), stop=(dt == 7))
                        v_aug = wk.tile([128, 2, 8, 65], BF16, tag="v_aug",
                                        bufs=2)
                        nc.scalar.copy(
                            v_aug[:, :, :, 0:64],
                            vps[:, :, :].rearrange("p t (h w) -> p t h w", h=8))
                        nc.gpsimd.memset(v_aug[:, :, :, 64:65], 1.0)

                        # ---- sim: 2-term rope accumulation ----
                        sps = psb.tile([128, 2, 512], F32, tag="ps",
                                       name=f"sps{c}")
                        # bank = h%2 so each PSUM bank sees a single PE
                        # tile_position (row-0/row-64 mixing in one bank
                        # crashes the device)
                        for h in range(H):
                            hb = 0 if force_base0 else 64 * (h % 2)
                            itm = (h % 4) // 2
                            kc_t = kcA if h < 4 else kcB
                            kg_t = kgA if h < 4 else kgB
                            for jg in range(2):
                                js = sub * TK + jg * 128
                                dst = sps[:, h % 2,
                                          (h // 2) * 128 + jg * 64:
                                          (h // 2) * 128 + jg * 64 + 64]
                                nc.tensor.matmul(
                                    dst,
                                    kc_t[hb:hb + 64, itm, js:js + 128],
                                    q_sb[hb:hb + 64, h // 2,
                                         cing * CS:(cing + 1) * CS],
                                    start=True, stop=False)
                                nc.tensor.matmul(
                                    dst,
                                    kg_t[hb:hb + 64, itm, js:js + 128],
                                    qr_sb[hb:hb + 64, h // 2,
                                          cing * CS:(cing + 1) * CS],
                                    start=False, stop=True)
                        expT = wk.tile([128, 2, 512], BF16, tag="expT", bufs=2)
                        nc.scalar.activation(expT, sps,
                                             mybir.ActivationFunctionType.Exp)
                        if stage == 2:
                            dbg = wk.tile([64, DIM], F32, tag="dbg", bufs=2)
                            nc.vector.tensor_copy(
                                dbg[:, 0:1024], expT[0:64, :, :].rearrange(
                                    "p jg hi -> p (jg hi)"))
                            nc.sync.dma_start(out=out[c * 64:(c + 1) * 64, :],
                                              in_=dbg)
                            continue
                        sub_state.append((sub, c, cing, expT, v_aug))

                    # drain the previous pair's out-projection PSUM while PE
                    # runs the o matmuls below (ACT is past this pair's exps)
                    while pending_out:
                        emit_drain(pending_out.pop())

                    for (sub, c, cing, expT, v_aug) in sub_state:
                        # ---- o matmuls: [64 i, 65] per head, bank-safe ----
                        ops_ = psb.tile([64, 2, 512], F32, tag="ps",
                                        name=f"ops{c}")
                        # per bank: null term opens the accumulation group
                        # (start=True zeroes + writes en*nullv for 4 heads via
                        # block-diag null-v), the 8 head/jg matmuls accumulate,
                        # the last one closes it
                        for half in range(2):
                            nc.tensor.matmul(
                                ops_[:, half, 0:260],
                                expn8[0:8, cing * CS:cing * CS + CS],
                                nullv_bf[:, half, :],
                                start=True, stop=False, skip_group_check=True)
                            for hh in range(4):
                                h = half * 4 + hh
                                dst = ops_[:, half, hh * 65:hh * 65 + 65]
                                for jg in range(2):
                                    nc.tensor.matmul(
                                        dst,
                                        expT[:, h % 2,
                                             (h // 2) * 128 + jg * 64:
                                             (h // 2) * 128 + jg * 64 + 64],
                                        v_aug[:, jg, h, :],
                                        start=False,
                                        stop=(hh == 3 and jg == 1),
                                        skip_group_check=True)

                        # ---- normalize into o_pair (recip on DVE, mul on Pool)
                        rcol = wk.tile([64, 2, 4], F32, tag="rcol", bufs=2)
                        nc.vector.reciprocal(rcol, ops_[:, :, 64:260:65])
                        ofull = ops_[:, :, :]
                        oview = bass.AP(
                            tensor=ofull.tensor, offset=ofull.offset,
                            ap=[ofull.ap[0], [512, 2], [65, 4], [1, 64]])
                        nc.vector.tensor_mul(
                            o_pair[sub * 64:(sub + 1) * 64, :, :].rearrange(
                                "p (half hh) w -> p half hh w", half=2),
                            oview,
                            rcol.unsqueeze(3).broadcast_to((64, 2, 4, 64)))

                    if stage == 3:
                        dbg = wk.tile([128, DIM], F32, tag="dbg", bufs=2)
                        nc.vector.tensor_copy(dbg[:, 0:512], o_pair[:, :, :]
                                              .rearrange("p h w -> p (h w)"))
                        nc.vector.memset(dbg[:, 512:], 0.0)
                        nc.sync.dma_start(out=out[p * 128:(p + 1) * 128, :],
                                          in_=dbg)
                        continue
                    if stage >= 4:
                        prev.update({"o_pair": o_pair, "p": p})

            # final pair's tail
            if prev:
                emit_trans(prev)
                emit_out(prev)
                emit_drain(prev)
            while pending_out:
                emit_drain(pending_out.pop())

    nc.compile()
    return nc


_CACHED_NC = None


def _get_nc():
    global _CACHED_NC
    if _CACHED_NC is None:
        _CACHED_NC = _build_bass()
    return _CACHED_NC


def _marshal(x, context, q_pos_emb, k_pos_emb, Wq, Wk, Wv, Wo, bo, null_k, null_v):
    """Host-side prep: causal shift, token-0 rope folding, weight/table prep."""
    # causal shift: drop first CP tokens of x, pad CP zeros at end
    xs = np.zeros_like(x)
    xs[:, : N - CP] = x[:, CP:]
    xc = np.ascontiguousarray(xs.reshape(BK, CS, DIM))

    # fold rope-q (nontrivial only at token 0 of each chunk) into x:
    # x~0 = x0 + Wq (Wq^T Wq)^-1 (R0 - I) Wq^T x0
    qpe63 = np.asarray(q_pos_emb[0, 0, CP], dtype=np.float64)    # [64]
    cos0, sin0 = np.cos(qpe63), np.sin(qpe63)
    RH = np.zeros((64, 64))
    for e in range(32):
        RH[e, e + 32] = -1.0
        RH[e + 32, e] = 1.0
    R0 = np.diag(cos0) + np.diag(sin0) @ RH                      # [64, 64]
    R0b = np.kron(np.eye(8), R0)                                 # [512, 512]
    Wq64 = np.asarray(Wq, dtype=np.float64)
    A = Wq64.T @ Wq64
    M = (R0b - np.eye(512)) @ Wq64.T                             # [512, 1024]
    X0 = xc[:, 0, :].astype(np.float64)                          # [BK, 1024]
    corr = (Wq64 @ np.linalg.solve(A, M @ X0.T)).T               # [BK, 1024]
    xc[:, 0, :] = (X0 + corr).astype(np.float32)

    ctx = context.reshape(BK, TK, DIM)

    # Wq extended: pre-scaled + 8 null-sim columns (Wq_h @ null_k_h)
    Wq_s = Wq * SCALE
    nullk = null_k.reshape(8, 64)
    wqn = np.stack([Wq_s[:, h * 64:(h + 1) * 64] @ nullk[h] for h in range(8)],
                   axis=1)                                       # [1024, 8]
    Wq_full = np.concatenate([Wq_s, wqn], axis=1).astype(BF)     # [1024, 520]

    # rope-k tables in (d_on_partition, j) layout, replicated across halves
    kpe = np.asarray(k_pos_emb[0, 0], dtype=np.float32)          # [128, 64]
    cosk = np.cos(kpe)                                           # [128, 64]
    sink = np.sin(kpe)
    ge = np.empty((128, 64), np.float32)
    ge[:, :32] = sink[:, 32:]
    ge[:, 32:] = -sink[:, :32]
    Tcos = np.ascontiguousarray(np.tile(cosk.T, (2, 1)))         # [128, 128]
    Tg = np.ascontiguousarray(np.tile(ge.T, (2, 1)))             # [128, 128]

    # block-diagonal null-v: nullv_bd[h, (h%4)*65 : +65] = [null_v_h, 1]
    # in the bank half h//4; zeros elsewhere (single-matmul null o-term)
    nullv_aug = np.zeros((8, 2, 260), np.float32)
    nv = null_v.reshape(8, 64)
    for h in range(8):
        nullv_aug[h, h // 4, (h % 4) * 65:(h % 4) * 65 + 64] = nv[h]
        nullv_aug[h, h // 4, (h % 4) * 65 + 64] = 1.0

    shared = {
        "Wq": Wq_full,
        "Wk": np.ascontiguousarray(Wk).astype(BF),
        "Wv": np.ascontiguousarray(Wv).astype(BF),
        "Wo": np.ascontiguousarray(Wo).astype(BF),
        "bo": np.ascontiguousarray(bo, dtype=np.float32),
        "Tcos": Tcos, "Tg": Tg,
        "nullv_aug": np.ascontiguousarray(nullv_aug.reshape(8, 520)),
    }
    in_maps = []
    for cix in range(N_CORES):
        sl = slice(cix * CPC, (cix + 1) * CPC)
        xT_c = np.ascontiguousarray(xc[sl].reshape(TQ, DIM).T.astype(BF))
        ctxT_c = np.ascontiguousarray(ctx[sl].reshape(TCTX, DIM).T.astype(BF))
        in_maps.append({"xT": xT_c, "ctxT": ctxT_c, **shared})
    return in_maps


def kernel(x, context, q_pos_emb, k_pos_emb, Wq, Wk, Wv, Wo, bo, null_k, null_v):
    x = np.asarray(x, dtype=np.float32)
    context = np.asarray(context, dtype=np.float32)
    q_pos_emb = np.asarray(q_pos_emb, dtype=np.float32)
    k_pos_emb = np.asarray(k_pos_emb, dtype=np.float32)
    Wq = np.asarray(Wq, dtype=np.float32)
    Wk = np.asarray(Wk, dtype=np.float32)
    Wv = np.asarray(Wv, dtype=np.float32)
    Wo = np.asarray(Wo, dtype=np.float32)
    bo = np.asarray(bo, dtype=np.float32)
    null_k = np.asarray(null_k, dtype=np.float32)
    null_v = np.asarray(null_v, dtype=np.float32)

    in_maps = _marshal(x, context, q_pos_emb, k_pos_emb, Wq, Wk, Wv, Wo, bo,
                       null_k, null_v)
    nc = _get_nc()
    res = run_bass_kernel_spmd(nc, in_maps, core_ids=list(range(N_CORES)))

    out_full = np.concatenate([res.results[c]["out"] for c in range(N_CORES)],
                              axis=0)                      # [BK*CS, DIM]
    o = out_full.reshape(B, K_CHUNKS * CS, DIM)
    final = np.concatenate(
        [np.zeros((B, CP, DIM), np.float32), o[:, : K_CHUNKS * CS - CP]], axis=1)
    return final
